# revision 3
# baseline (speedup 1.0000x reference)
"""Doc2vec-style embedding lookup kernel for 8 Trainium2 NeuronCores.

Computation (per batch row b):
    h[b,:]      = D[docs[b],:] + sum_c W[ctxs[b,c],:]          # [B, DIM]
    scores[b,k] = sum_d h[b,d] * WP[d, targets[b,k]]           # [B, K1]

Strategy: pure batch data-parallel over 8 cores (2048 rows each), tables
replicated.  On the host we fold W, D and WP^T into ONE row-padded table
(rows padded 100 -> 128 floats = 512B so every gather descriptor moves a
full 512B line) and pack all per-row indices into a single [B, 15] int32
array (8 ctx, 1 doc, 6 target indices, pre-offset into the fused table).
On device each core does 4 big indirect-DMA gathers (one per 4 batch
tiles of 128 rows), a strided 9-way vector reduce for h, and 6 fused
multiply-reduce ops per tile for the scores.
"""

import sys

sys.path.insert(0, "/opt/trn_rl_repo")

import numpy as np

# ---- problem constants (hardcoded; kernel.py must be self-contained) ----
B = 16384  # batch
CTX = 8  # context words per row
K1 = 6  # targets per row
DIM = 100  # embedding dim
NW = 200001  # word rows (incl. padding row)
ND = 1000000  # doc rows
NCORES = 8
BPC = B // NCORES  # 2048 batch rows per core
P = 128  # SBUF partitions
TILES = BPC // P  # 16 tiles of 128 rows per core
JPT = CTX + 1 + K1  # 15 gathered rows per batch row
DPAD = 128  # padded row length (512B)
MEGA = 4  # batch tiles per indirect gather
NMEGA = TILES // MEGA

_CACHE: dict = {}


def _build_program():
    import concourse.bass as bass
    import concourse.bacc as bacc
    import concourse.mybir as mybir
    import concourse.tile as tile

    nc = bacc.Bacc("TRN2", target_bir_lowering=False, debug=False,
                   num_devices=NCORES)
    idx_d = nc.dram_tensor("idx", [BPC, JPT], mybir.dt.int32,
                           kind="ExternalInput")
    tab_d = nc.dram_tensor("table", [NW + ND + NW, DPAD], mybir.dt.float32,
                           kind="ExternalInput")
    out_d = nc.dram_tensor("scores", [BPC, K1], mybir.dt.float32,
                           kind="ExternalOutput")

    with tile.TileContext(nc) as tc:
        with tc.tile_pool(name="sb", bufs=1) as sb, \
             tc.tile_pool(name="gp", bufs=2) as gp, \
             tc.tile_pool(name="scr", bufs=4) as scr:
            idx_sb = sb.tile([P, TILES * JPT], mybir.dt.int32)
            nc.sync.dma_start(
                out=idx_sb[:].rearrange("p (t j) -> p t j", t=TILES),
                in_=idx_d.ap().rearrange("(t p) j -> p t j", p=P),
            )
            scores_sb = sb.tile([P, TILES * K1], mybir.dt.float32)
            nrows = NW + ND + NW
            for t in range(TILES):
                # HW indirect DMA supports ONE offset per partition, so we
                # gather the 15 rows of this 128-row batch tile with 15 ops.
                G = gp.tile([P, JPT * DPAD], mybir.dt.float32)
                for j in range(JPT):
                    nc.gpsimd.indirect_dma_start(
                        out=G[:, j * DPAD:(j + 1) * DPAD],
                        out_offset=None,
                        in_=tab_d.ap(),
                        in_offset=bass.IndirectOffsetOnAxis(
                            ap=idx_sb[:, t * JPT + j:t * JPT + j + 1],
                            axis=0,
                        ),
                        bounds_check=nrows - 1,
                        oob_is_err=False,
                    )
                G3 = G[:].rearrange("p (j d) -> p d j", j=JPT, d=DPAD)
                h = scr.tile([P, DIM], mybir.dt.float32, tag="h")
                # h = sum of the 8 ctx rows + 1 doc row (slabs j=0..8)
                nc.vector.tensor_reduce(
                    out=h[:], in_=G3[:, 0:DIM, 0:CTX + 1],
                    axis=mybir.AxisListType.X, op=mybir.AluOpType.add,
                )
                # prod[p, k, d] = h[p, d] * tgt_k[p, d]; then reduce over d
                prod = scr.tile([P, K1 * DIM], mybir.dt.float32, tag="prod")
                tgt = G[:].rearrange("p (j d) -> p j d", j=JPT)
                nc.vector.tensor_tensor(
                    out=prod[:].rearrange("p (k d) -> p k d", k=K1),
                    in0=tgt[:, CTX + 1:JPT, 0:DIM],
                    in1=h[:].unsqueeze(1).to_broadcast([P, K1, DIM]),
                    op=mybir.AluOpType.mult,
                )
                nc.vector.tensor_reduce(
                    out=scores_sb[:, t * K1:(t + 1) * K1],
                    in_=prod[:].rearrange("p (k d) -> p k d", k=K1),
                    axis=mybir.AxisListType.X, op=mybir.AluOpType.add,
                )
            nc.sync.dma_start(
                out=out_d.ap().rearrange("(t p) k -> p t k", p=P),
                in_=scores_sb[:].rearrange("p (t k) -> p t k", t=TILES),
            )
    nc.compile()
    return nc


def _get_program():
    if "nc" not in _CACHE:
        _CACHE["nc"] = _build_program()
    return _CACHE["nc"]


def _pack_inputs(ctxs, docs, targets, D, W, WP):
    """Fuse tables into one 512B-row table; pack indices to [B, 15] int32."""
    table = np.zeros((NW + ND + NW, DPAD), dtype=np.float32)
    table[:NW, :DIM] = np.asarray(W, dtype=np.float32)
    table[NW:NW + ND, :DIM] = np.asarray(D, dtype=np.float32)
    table[NW + ND:, :DIM] = np.asarray(WP, dtype=np.float32).T
    idx = np.empty((B, JPT), dtype=np.int32)
    idx[:, :CTX] = np.asarray(ctxs, dtype=np.int32)
    idx[:, CTX] = np.asarray(docs, dtype=np.int32) + NW
    idx[:, CTX + 1:] = np.asarray(targets, dtype=np.int32) + (NW + ND)
    return table, idx


def kernel(ctxs, docs, targets, D, W, WP, _trace=False):
    from concourse.bass_utils import run_bass_kernel_spmd

    table, idx = _pack_inputs(ctxs, docs, targets, D, W, WP)
    nc = _get_program()
    in_maps = [
        {"idx": np.ascontiguousarray(idx[c * BPC:(c + 1) * BPC]),
         "table": table}
        for c in range(NCORES)
    ]
    res = run_bass_kernel_spmd(nc, in_maps, core_ids=list(range(NCORES)),
                               trace=_trace)
    out = np.concatenate([res.results[c]["scores"] for c in range(NCORES)],
                         axis=0)
    if _trace:
        return out, res
    return out
